# revision 17
# baseline (speedup 1.0000x reference)
"""Trainium2 Bass kernel for nn_LowPassCTRNN.

Model (per reference):
  xd = einsum('bti,ri->btr', input, Wrx) + bx
  step: f = max(tanh(r @ Wrr.T + xd_t), 0); r' = 0.9 r + 0.1 f + 0.1 br_t
  rstore = stacked r over t;  y = einsum('btr,or->bto', rstore, Wyr) + by
  returns (y, rstore)

Sharding: data-parallel over batch B=64 across 8 cores (B_local=8).
Each core runs the full T=1000 recurrence on its batch slice.

Device design (per core):
  - state r transposed: [128 part, 4 chunks, 8 batch]; matmul state in fp16
  - per step: 2 identity matmuls accumulate xd(fp16) into PSUM, then 16
    Wrr^T(fp16) matmuls (stationary tiles, fast-weight-load) accumulate
    z = r @ Wrr.T + xd into two PSUM half-banks
  - serial chain per half: ACT tanh (PSUM src) -> DVE (max,*0.1) ->
    DVE add -> fp16 state ring buffer (feeds next step's matmuls)
  - GPSIMD (Pool) off-chain: P = 0.9 r + 0.1 br and the fp32 state
    duplicate that becomes the rstore output (full fp32 state accuracy)
"""

import os
import numpy as np

B, T_FULL, DIN, DREC, DOUT = 64, 1000, 128, 512, 64
NCORES = 8
BL = B // NCORES  # 8 batch per core
NCH = DREC // 128  # 4 chunks

_CACHE = {}


def _mm_schedule():
    """(m, j, stop) emission order for the 16 Wrr matmuls of one step.

    Half A = tiles {0,1}, half B = {2,3}. Early slots consume early-ready
    state chunks {0,1}; chunks {2,3} are consumed from slot 5 onward.
    """
    order = [
        (0, 0), (0, 1), (1, 1), (1, 0), (0, 2), (0, 3), (1, 3), (1, 2),
        (2, 0), (2, 1), (3, 1), (3, 0), (2, 2), (2, 3), (3, 3), (3, 2),
    ]
    last_slot = {}
    for i, (m, j) in enumerate(order):
        last_slot[m] = i
    return [(m, j, i == last_slot[m]) for i, (m, j) in enumerate(order)]


def _build(T, U):
    """Build the per-core Bass program. T = timesteps, U = steps per DMA block."""
    import concourse.bacc as bacc
    import concourse.tile as tile
    import concourse.mybir as mybir

    assert T % U == 0
    TH = T // 2  # phase-1/3 column half
    f32, f16 = mybir.dt.float32, mybir.dt.float16
    AF = mybir.ActivationFunctionType
    OP = mybir.AluOpType
    sched = _mm_schedule()

    nc = bacc.Bacc(None, target_bir_lowering=False)

    # ---- I/O ----
    inputT = nc.dram_tensor("inputT", [128, BL, T], f16, kind="ExternalInput")
    brs = nc.dram_tensor("brs", [NCH, 128, BL, T], f32, kind="ExternalInput")
    wrr = nc.dram_tensor("wrr", [128, NCH, NCH, 128], f16, kind="ExternalInput")
    wrx = nc.dram_tensor("wrx", [128, NCH, 128], f16, kind="ExternalInput")
    bxd = nc.dram_tensor("bx", [128, NCH], f32, kind="ExternalInput")
    wyr = nc.dram_tensor("wyr", [128, NCH, DOUT], f16, kind="ExternalInput")
    iden = nc.dram_tensor("iden", [128, 128], f16, kind="ExternalInput")
    r0rep = nc.dram_tensor("r0rep", [128, NCH, BL], f32, kind="ExternalInput")
    rstore = nc.dram_tensor("rstore", [NCH, 128, BL, T], f32, kind="ExternalOutput")
    yT = nc.dram_tensor("yT", [DOUT, BL, T], f32, kind="ExternalOutput")
    xd = nc.dram_tensor("xd", [NCH, 128, BL, T], f16)  # internal scratch
    rst16 = nc.dram_tensor("rst16", [NCH, 128, BL, T], f16)  # internal scratch

    with tile.TileContext(nc) as tc:
        with (
            tc.tile_pool(name="singles", bufs=1) as singles,
            tc.tile_pool(name="blocks", bufs=4) as blocks,
            tc.tile_pool(name="state", bufs=8) as state,
            tc.tile_pool(name="evac", bufs=4) as evac,
        ):
            # ---- constants into SBUF ----
            wrr_sb = singles.tile([128, NCH, NCH, 128], f16)
            nc.sync.dma_start(out=wrr_sb[:], in_=wrr[:])
            wrx_sb = singles.tile([128, NCH, 128], f16)
            nc.sync.dma_start(out=wrx_sb[:], in_=wrx[:])
            bx_sb = singles.tile([128, NCH], f32)
            nc.sync.dma_start(out=bx_sb[:], in_=bxd[:])
            wyr_sb = singles.tile([128, NCH, DOUT], f16)
            nc.sync.dma_start(out=wyr_sb[:], in_=wyr[:])
            iden_sb = singles.tile([128, 128], f16)
            nc.sync.dma_start(out=iden_sb[:], in_=iden[:])
            inT_sb = singles.tile([128, BL, T], f16)
            nc.sync.dma_start(out=inT_sb[:], in_=inputT[:])
            r0_sb = singles.tile([128, NCH, BL], f32)
            nc.sync.dma_start(out=r0_sb[:], in_=r0rep[:])

            # ---- phase 1: xd[m, p, b, t] = (input @ Wrx^T + bx), fp16 out ----
            with tc.tile_pool(name="ps1", bufs=4, space="PSUM") as ps1:
                for m in range(NCH):
                    for b in range(BL):
                        for h in range(2):
                            pp = ps1.tile([128, TH], f32, tag="pp1")
                            nc.tensor.matmul(
                                pp[:],
                                wrx_sb[:, m, :],
                                inT_sb[:, b, h * TH:(h + 1) * TH],
                                start=True, stop=True,
                            )
                            ev = evac.tile([128, TH], f16, tag="ev1")
                            if (b + h) % 2 == 0:
                                nc.vector.tensor_scalar(
                                    ev[:], pp[:], bx_sb[:, m:m + 1], None, OP.add)
                            else:
                                nc.scalar.activation(
                                    ev[:], pp[:], AF.Identity, bias=bx_sb[:, m:m + 1])
                            nc.sync.dma_start(
                                out=xd[m, :, b, h * TH:(h + 1) * TH], in_=ev[:])

            # ---- phase 2: recurrence ----
            with tc.tile_pool(name="psr", bufs=4, space="PSUM") as psr:
                # initial state
                r16_init = state.tile([128, NCH, BL], f16, tag="r16i")
                nc.vector.tensor_copy(r16_init[:], r0_sb[:])
                r16_prev = r16_init[:]
                rprev_f32 = r0_sb[:]

                n_sb = T // U
                cut = globals().get("_LOOP_CUT")
                if cut:  # timing experiments only: truncate the recurrence
                    n_sb = min(n_sb, cut // U)
                for sb_i in range(n_sb):
                    t0 = sb_i * U
                    xd_blk = blocks.tile([128, NCH, BL, U], f16, tag="xd")
                    brs_blk = blocks.tile([128, NCH, BL, U], f32, tag="brs")
                    for j in range(NCH):
                        nc.sync.dma_start(
                            out=xd_blk[:, j, :, :], in_=xd[j, :, :, t0:t0 + U])
                        nc.sync.dma_start(
                            out=brs_blk[:, j, :, :], in_=brs[j, :, :, t0:t0 + U])
                    rout_blk = blocks.tile([128, NCH, BL, U], f32, tag="rout")
                    r16_blk = blocks.tile([128, NCH, BL, U], f16, tag="r16b")

                    for ul in range(U):
                        # P = 0.9*r + brs on Pool (off the serial chain)
                        p0 = state.tile([128, NCH, BL], f32, tag="p0")
                        nc.gpsimd.tensor_scalar(p0[:], rprev_f32, 0.9, None, OP.mult)
                        p1 = state.tile([128, NCH, BL], f32, tag="p1")
                        nc.gpsimd.tensor_tensor(
                            p1[:], p0[:], brs_blk[:, :, :, ul], OP.add)

                        zA = psr.tile([128, 2, BL], f32, tag="zA")
                        zB = psr.tile([128, 2, BL], f32, tag="zB")
                        fs = state.tile([128, NCH, BL], f32, tag="fs")

                        for h, z in ((0, zA), (1, zB)):
                            sl = slice(2 * h, 2 * h + 2)
                            # xd into PSUM via identity matmul (start clears
                            # the bank), then this half's 8 Wrr matmuls
                            # accumulate z = r @ Wrr^T + xd on top.
                            nc.tensor.matmul(
                                z[:], iden_sb[:], xd_blk[:, sl, :, ul],
                                start=True, stop=False, skip_group_check=True)
                            for (m, j, lst) in sched[8 * h:8 * h + 8]:
                                nc.tensor.matmul(
                                    z[:, m % 2, :],
                                    wrr_sb[:, j, m, :],
                                    r16_prev[:, j, :],
                                    start=False, stop=lst,
                                    skip_group_check=True)
                            # chain: ACT tanh -> Pool relu*0.1 -> DVE add(f16)
                            thh = state.tile([128, 2, BL], f32, tag="th%d" % h)
                            nc.scalar.activation(thh[:], z[:], AF.Tanh)
                            nc.gpsimd.tensor_scalar(
                                fs[:, sl, :], thh[:], 0.0, 0.1, OP.max, OP.mult)
                            nc.vector.tensor_tensor(
                                r16_blk[:, sl, :, ul], p1[:, sl, :],
                                fs[:, sl, :], OP.add)
                        # fp32 state duplicate -> rstore (Pool, off-chain)
                        nc.gpsimd.tensor_tensor(
                            rout_blk[:, :, :, ul], p1[:], fs[:], OP.add)
                        r16_prev = r16_blk[:, :, :, ul]
                        rprev_f32 = rout_blk[:, :, :, ul]

                    for j in range(NCH):
                        nc.sync.dma_start(
                            out=rstore[j, :, :, t0:t0 + U],
                            in_=rout_blk[:, j, :, :])
                        nc.sync.dma_start(
                            out=rst16[j, :, :, t0:t0 + U],
                            in_=r16_blk[:, j, :, :])

            # ---- phase 3: y^T[o, b, t] = r @ Wyr^T (fp16 matmul) ----
            with tc.tile_pool(name="ps3", bufs=4, space="PSUM") as ps3:
                for b in range(BL):
                    for h in range(2):
                        rstb = blocks.tile([128, NCH, TH], f16, tag="rst")
                        nc.sync.dma_start(
                            out=rstb[:],
                            in_=rst16[:, :, b, h * TH:(h + 1) * TH]
                            .rearrange("j p t -> p j t"),
                        )
                        py = ps3.tile([DOUT, TH], f32, tag="pp3")
                        for m in range(NCH):
                            nc.tensor.matmul(
                                py[:], wyr_sb[:, m, :], rstb[:, m, :],
                                start=(m == 0), stop=(m == NCH - 1),
                            )
                        evy = evac.tile([DOUT, TH], f32, tag="ev3")
                        if h == 0:
                            nc.vector.tensor_copy(evy[:], py[:])
                        else:
                            nc.scalar.copy(evy[:], py[:])
                        nc.sync.dma_start(
                            out=yT[:, b, h * TH:(h + 1) * TH], in_=evy[:])

    nc.finalize()
    return nc


def _get_nc(T, U):
    key = (T, U)
    if key not in _CACHE:
        _CACHE[key] = _build(T, U)
    return _CACHE[key]


def kernel(input, brneverlearn, Wrx, bx, Wrr, Wyr, by, r0,
           T=None, U=None, _return_res=False):
    from concourse.bass_utils import run_bass_kernel_spmd

    input = np.asarray(input, dtype=np.float32)
    brneverlearn = np.asarray(brneverlearn, dtype=np.float32)
    Wrx = np.asarray(Wrx, dtype=np.float32)
    bx = np.asarray(bx, dtype=np.float32)
    Wrr = np.asarray(Wrr, dtype=np.float32)
    Wyr = np.asarray(Wyr, dtype=np.float32)
    by = np.asarray(by, dtype=np.float32)
    r0 = np.asarray(r0, dtype=np.float32)

    T = T or input.shape[1]
    U = U or int(os.environ.get("CTRNN_U", 25))
    nc = _get_nc(T, U)

    # shared (replicated) weights
    wrr_h = np.ascontiguousarray(
        Wrr.T.reshape(NCH, 128, NCH, 128).transpose(1, 0, 2, 3), dtype=np.float16)
    wrx_h = np.ascontiguousarray(Wrx.T.reshape(128, NCH, 128), dtype=np.float16)
    bx_h = np.ascontiguousarray(bx.reshape(NCH, 128).T, dtype=np.float32)
    wyr_h = np.ascontiguousarray(
        Wyr.T.reshape(NCH, 128, DOUT).transpose(1, 0, 2), dtype=np.float16)
    iden_h = np.eye(128, dtype=np.float16)
    r0_h = np.ascontiguousarray(
        np.broadcast_to(r0.reshape(NCH, 128).T[:, :, None], (128, NCH, BL)),
        dtype=np.float32)

    in_maps = []
    for c in range(NCORES):
        sl = slice(c * BL, (c + 1) * BL)
        inT = np.ascontiguousarray(
            input[sl, :T].transpose(2, 0, 1), dtype=np.float16)
        brs_h = np.ascontiguousarray(
            (0.1 * brneverlearn[sl, :T]).transpose(2, 0, 1), dtype=np.float32
        ).reshape(NCH, 128, BL, T)
        in_maps.append({
            "inputT": inT, "brs": brs_h, "wrr": wrr_h, "wrx": wrx_h,
            "bx": bx_h, "wyr": wyr_h, "iden": iden_h, "r0rep": r0_h,
        })

    res = run_bass_kernel_spmd(nc, in_maps, core_ids=list(range(NCORES)))

    y = np.empty((B, T, DOUT), dtype=np.float32)
    rs = np.empty((B, T, DREC), dtype=np.float32)
    for c in range(NCORES):
        sl = slice(c * BL, (c + 1) * BL)
        out = res.results[c]
        rs[sl] = out["rstore"].transpose(2, 3, 0, 1).reshape(BL, T, DREC)
        y[sl] = out["yT"].transpose(1, 2, 0) + by
    if _return_res:
        return (y, rs), res
    return (y, rs)


def estimate_time_ns(T=T_FULL, U=None):
    """Cost-model timeline estimate for one core's program (ns)."""
    try:
        from concourse.timeline_sim import TimelineSim
        U = U or int(os.environ.get("CTRNN_U", 25))
        nc = _get_nc(T, U)
        sim = TimelineSim(nc)
        return int(sim.simulate())
    except Exception as e:
        print("TimelineSim failed:", e)
        return None
